# revision 42
# baseline (speedup 1.0000x reference)
"""GQA attention with QK-norm for Trainium2, sharded over 8 NeuronCores.

Problem: B=2, T=2048, D=2048, H=16 query heads, KVH=4 kv heads, dk=128.
    Q = q @ Wq.T ; K = k @ Wk.T ; V = v @ Wv.T  (per batch)
    Q = g * l2norm(Q, per head) ; K = l2norm(K, per head)
    out = softmax(causal(Q K^T / sqrt(dk))) V @ Wo.T

Sharding: core c = 4*b + gi handles batch b and kv-head group gi
(4 query heads + 1 kv head). Each core computes a row-shard of the
output projection (O^T partial over its 512 head-dims); the host sums
the 4 partials per batch. No device collectives.

On-core layout is feature-major ("transposed") throughout; softmax
numerator exp(S^T) needs no max-subtraction because QK-norm bounds
|scores| <= g/sqrt(dk) ~ 0.674.

v2 schedule (PE-density rewrite):
  phase A: K-proj -> K-norm -> Q-proj (tb-outer) -> Q-norms -> V-proj
  -> V-transpose.  Norm chains hide under the next projection's
  matmuls; broadcasts use gpsimd partition_broadcast (no HBM hop).
  phase B: (qb, h) columns are software-pipelined one head ahead:
  PE stream is s1(i), s1(i+1), s2(i), s1(i+2), s2(i+1), ... where
  s1 = scores matmuls (exp on ACT trails one head behind) and
  s2 = rowsum + AV matmuls off the finished exp strip, so the PE
  never waits on the scalar engine.  Diagonal k-tiles are narrowed
  to their live (causal) columns in scores/rowsum/AV/exp.  The
  out-projection of each t-block is emitted two pipeline slots after
  the block's last head, hiding the rowsum-reciprocal chain.
"""

import math
import os
import sys

for _p in ("/opt/trn_rl_repo",):
    if _p not in sys.path:
        sys.path.append(_p)

import numpy as np
from concourse import bacc, bass_isa, mybir, tile
from concourse.bass_utils import run_bass_kernel_spmd
from concourse.masks import make_identity

B, T, D, H, KVH, DK = 2, 2048, 2048, 16, 4, 128
HPG = H // KVH          # query heads per core (group)
E = HPG * DK            # 512: q-head dims per core
P = 128
TB = 4                  # t blocks of 512
NT = T // P             # 16 tiles of 128 along T
ND = D // P             # 16 contraction tiles
f32 = mybir.dt.float32
f32r = mybir.dt.float32r
bf16 = mybir.dt.bfloat16
AF = mybir.ActivationFunctionType
EPS2 = 1e-24

MM_DT = {"f32r": f32r, "bf16": bf16, "f32": f32}[
    os.environ.get("ATTN_DT", "bf16")]
ST_DT = bf16 if MM_DT == bf16 else f32


def _ld(x):
    return x.bitcast(MM_DT) if MM_DT is f32r else x


def build_kernel():
    nc = bacc.Bacc(None, target_bir_lowering=False)

    # host-pre-tiled inputs (see make_in_maps): every DMA below reads
    # contiguous multi-KB rows.
    qTt = nc.declare_dram_parameter("qTt", [TB, P, ND * 512], ST_DT,
                                    isOutput=False)
    kT = nc.declare_dram_parameter("kT", [D, T], ST_DT, isOutput=False)
    vT = nc.declare_dram_parameter("vT", [D, T], ST_DT, isOutput=False)
    wqt = nc.declare_dram_parameter("wqt", [P, ND * E], ST_DT, isOutput=False)
    wkt = nc.declare_dram_parameter("wkt", [P, ND * DK], ST_DT,
                                    isOutput=False)
    wvt = nc.declare_dram_parameter("wvt", [P, ND * DK], ST_DT,
                                    isOutput=False)
    wot = nc.declare_dram_parameter("wot", [P, HPG * D], ST_DT,
                                    isOutput=False)
    gs16 = nc.declare_dram_parameter("gs16", [NT, HPG * P], f32,
                                     isOutput=False)
    outT = nc.declare_dram_parameter("outT", [D, T], bf16, isOutput=True)

    n2_dram = nc.dram_tensor("n2_dram", [HPG + 1, T], f32)
    c_dram = nc.dram_tensor("c_dram", [HPG + 1, T], bf16)

    from contextlib import ExitStack

    with tile.TileContext(nc) as tc:
        with ExitStack() as outer:
            const = outer.enter_context(tc.tile_pool(name="const", bufs=1))
            persist = outer.enter_context(tc.tile_pool(name="persist", bufs=1))

            ident = const.tile([P, P], f32, tag="ident")
            make_identity(nc, ident[:])
            ones_f32 = const.tile([P, 1], f32, tag="ones_f32")
            nc.vector.memset(ones_f32[:], 1.0)
            ones = const.tile([P, 1], MM_DT, tag="ones")
            nc.vector.tensor_copy(ones[:], ones_f32[:])
            gs_sb = const.tile([NT, HPG * P], f32, tag="gs")
            nc.sync.dma_start(gs_sb[:], gs16[:])
            eps16 = const.tile([NT, 1], f32, tag="eps16")
            nc.vector.memset(eps16[:], EPS2)
            # causal additive mask: M[p, c] = 0 iff c >= p + 384 else -30;
            # added to the scores psum via an identity matmul so exp()
            # produces an already-masked strip. Live part of diagonal
            # k-tile: M[:, 384 : 384+W].
            identb = const.tile([P, P], MM_DT, tag="identb")
            nc.vector.tensor_copy(identb[:], ident[:])
            trimat = const.tile([P, 896], MM_DT, tag="trimat")
            nc.vector.memset(trimat[:], 0.0)
            nc.gpsimd.affine_select(
                out=trimat[:], in_=trimat[:],
                compare_op=mybir.AluOpType.is_ge,
                fill=-30.0, base=-384,
                pattern=[[1, 896]], channel_multiplier=-1,
            )

            # per-(head, t-block) tiles: slice-precise dependencies so a
            # late norm-scale write never false-blocks unrelated readers
            qts = [[persist.tile([P, 512], MM_DT, tag=f"qt{h}_{tb}",
                                 name=f"qt{h}_{tb}")
                    for tb in range(TB)] for h in range(HPG)]
            kt_sb = persist.tile([P, T], MM_DT, tag="kt")
            vtm_sb = persist.tile([P, T], MM_DT, tag="vtm")
            yts = [[persist.tile([P, 512], MM_DT, tag=f"yt{h}_{tb}",
                                 name=f"yt{h}_{tb}")
                    for tb in range(TB)] for h in range(HPG)]

            # ---------------- phase A: projections + norms ----------------
            with ExitStack() as pa:
                wpool = pa.enter_context(tc.tile_pool(name="wpool", bufs=1))
                actsq = pa.enter_context(tc.tile_pool(name="actsq", bufs=2))
                actskv = pa.enter_context(tc.tile_pool(name="actskv", bufs=6))
                actsv = pa.enter_context(tc.tile_pool(name="actsv", bufs=5))
                scratch = pa.enter_context(tc.tile_pool(name="scratch",
                                                        bufs=2))
                bcast = pa.enter_context(tc.tile_pool(name="bcast", bufs=8))
                bckp = pa.enter_context(tc.tile_pool(name="bckp", bufs=1))
                rows = pa.enter_context(tc.tile_pool(name="rows", bufs=4))
                smal = pa.enter_context(tc.tile_pool(name="smal", bufs=2))
                psA = pa.enter_context(
                    tc.tile_pool(name="psA", bufs=6, space="PSUM"))
                psTP = pa.enter_context(
                    tc.tile_pool(name="psTP", bufs=2, space="PSUM"))

                def proj_kv(src_dram, w_sb, dst_fn, pool):
                    # k/v chunks alternate between the sync and scalar HWDGE
                    # queues so the stream pulls at 2x single-queue rate.
                    accs = [psA.tile([P, 512], f32, tag="proj",
                                     name=f"acc{_t}") for _t in range(TB)]
                    for n in range(ND):
                        a = pool.tile([P, T], MM_DT, tag="akv")
                        eng = nc.sync if n % 2 == 0 else nc.scalar
                        eng.dma_start(
                            a[:], _ld(src_dram[n * P:(n + 1) * P, :]))
                        for tb in range(TB):
                            nc.tensor.matmul(
                                accs[tb][:],
                                w_sb[:, n * DK:(n + 1) * DK],
                                a[:, tb * 512:(tb + 1) * 512],
                                start=(n == 0), stop=(n == ND - 1))
                    for tb in range(TB):
                        dst_fn(tb, accs[tb])

                def norm_rowsums(xt_tb, idx, tb):
                    """Partition sums of xt_tb^2 [128, 512] via a ones-matmul,
                    staged to n2_dram for the per-head tail."""
                    sq = scratch.tile([P, 512], MM_DT, tag="sqs")
                    nc.vector.tensor_mul(sq[:], xt_tb, xt_tb)
                    ps = psTP.tile([1, 512], f32, tag="tp")
                    nc.tensor.matmul(ps[:], ones[:], sq[:],
                                     start=True, stop=True)
                    n2row = rows.tile([1, 512], f32, tag="n2row")
                    nc.vector.tensor_copy(n2row[:], ps[:])
                    nc.sync.dma_start(
                        n2_dram[idx:idx + 1, tb * 512:(tb + 1) * 512],
                        n2row[:])

                def rsqrt_chain(n2c, shp, gs_ap):
                    """y = rsqrt(n2c) (* gain), one Newton step, bf16 out."""
                    sq_c = smal.tile(shp, f32, tag="sqc")
                    nc.scalar.activation(sq_c[:], n2c[:], AF.Sqrt,
                                         bias=eps16[0:shp[0], :])
                    y0 = smal.tile(shp, f32, tag="y0")
                    nc.vector.reciprocal_approx_fast(y0[:], sq_c[:])
                    t1 = smal.tile(shp, f32, tag="t1")
                    nc.vector.tensor_mul(t1[:], y0[:], y0[:])
                    nc.vector.tensor_mul(t1[:], t1[:], n2c[:])
                    nc.vector.tensor_scalar(
                        out=t1[:], in0=t1[:], scalar1=-0.5, scalar2=1.5,
                        op0=mybir.AluOpType.mult, op1=mybir.AluOpType.add)
                    nc.vector.tensor_mul(y0[:], y0[:], t1[:])
                    if gs_ap is not None:
                        nc.vector.tensor_mul(y0[:], y0[:], gs_ap)
                    y0c = smal.tile(shp, bf16, tag="y0c")
                    nc.vector.tensor_copy(y0c[:], y0[:])
                    return y0c

                # --- Q-norm tail, software-pipelined per t-block ---------
                def q_gather(tb):
                    n2c = smal.tile([TB, HPG * P], f32, tag="n2c")
                    for h in range(HPG):
                        nc.sync.dma_start(
                            n2c[:, h * P:(h + 1) * P],
                            n2_dram[h, tb * 512:(tb + 1) * 512]
                            .rearrange("(c p) -> c p", p=P))
                    return n2c

                def q_chain(tb, n2c):
                    # gain is per-head (same for every chunk row): rows 0:4
                    y0c = rsqrt_chain(n2c, [TB, HPG * P], gs_sb[0:4, :])
                    for h in range(HPG):
                        nc.sync.dma_start(
                            c_dram[h, tb * 512:(tb + 1) * 512]
                            .rearrange("(c p) -> c p", p=P),
                            y0c[:, h * P:(h + 1) * P])

                def q_bc_issue(tb):
                    bcs = []
                    for h in range(HPG):
                        bc = bcast.tile([P, 512], bf16, tag="bc")
                        nc.sync.dma_start(
                            bc[:], c_dram[h:h + 1, tb * 512:(tb + 1) * 512]
                            .to_broadcast((P, 512)))
                        bcs.append(bc)
                    return bcs

                def q_mul(tb, bcs):
                    for h in range(HPG):
                        nc.vector.tensor_mul(qts[h][tb][:], qts[h][tb][:],
                                             bcs[h][:])

                # K first: wk rides the scalar queue while the first kT
                # chunk rides sync, so both land immediately; its norm
                # chain hides under the V projection.
                wk_sb = wpool.tile([P, ND * DK], MM_DT, tag="wk")
                nc.scalar.dma_start(wk_sb[:], _ld(wkt[:]))
                proj_kv(kT, wk_sb,
                        lambda tb, ps: nc.any.tensor_copy(
                            kt_sb[:, tb * 512:(tb + 1) * 512], ps[:]),
                        actskv)
                for tb in range(TB):
                    norm_rowsums(kt_sb[:, tb * 512:(tb + 1) * 512], HPG, tb)
                n2k = smal.tile([NT, P], f32, tag="n2c")
                nc.sync.dma_start(
                    n2k[:], n2_dram[HPG, :].rearrange("(c p) -> c p", p=P))
                y0k = rsqrt_chain(n2k, [NT, P], None)
                nc.sync.dma_start(
                    c_dram[HPG, :].rearrange("(c p) -> c p", p=P), y0k[:])
                bck = bckp.tile([P, T], bf16, tag="bck")
                nc.sync.dma_start(
                    bck[:], c_dram[HPG:HPG + 1, :].to_broadcast((P, T)))
                nc.vector.tensor_mul(kt_sb[:], kt_sb[:], bck[:])

                # V second: transposed per t-block as each psum drains.
                wv_sb = wpool.tile([P, ND * DK], MM_DT, tag="wv")
                nc.scalar.dma_start(wv_sb[:], _ld(wvt[:]))
                # wq prefetched during the V projection
                wq_sb = wpool.tile([P, ND * E], MM_DT, tag="wq")
                nc.sync.dma_start(wq_sb[:], _ld(wqt[:]))

                def v_stage(tb, ps):
                    vs = scratch.tile([P, 512], f32, tag="vstg")
                    nc.any.tensor_copy(vs[:], ps[:])
                    for j in range(4):
                        n = 4 * tb + j
                        tp = psTP.tile([P, P], f32, tag="tp")
                        nc.tensor.transpose(
                            tp[:], vs[:, j * P:(j + 1) * P], ident[:])
                        nc.vector.tensor_copy(
                            vtm_sb[:, n * P:(n + 1) * P], tp[:])
                proj_kv(vT, wv_sb, v_stage, actsv)

                # Q last, tb-outer; per-tb norm tails pipelined 1-2 blocks
                # behind so every chain input has its DMA latency covered.
                NH = ND // 2
                gathers, bcs_pend = {}, {}
                for tb in range(TB):
                    if tb >= 2:
                        bcs_pend[tb - 2] = q_bc_issue(tb - 2)
                    accs = [psA.tile([P, 512], f32, tag="proj",
                                     name=f"qacc{_h}") for _h in range(HPG)]
                    for half in range(2):
                        a = actsq.tile([P, NH * 512], MM_DT, tag="acts")
                        nc.sync.dma_start(
                            a[:], _ld(qTt[tb, :, half * NH * 512:
                                          (half + 1) * NH * 512]))
                        for h in range(HPG):
                            for nn in range(NH):
                                n = half * NH + nn
                                nc.tensor.matmul(
                                    accs[h][:],
                                    wq_sb[:,
                                          n * E + h * P:n * E + (h + 1) * P],
                                    a[:, nn * 512:(nn + 1) * 512],
                                    start=(n == 0), stop=(n == ND - 1))
                    for h in range(HPG):
                        nc.vector.tensor_copy(qts[h][tb][:], accs[h][:])
                        norm_rowsums(qts[h][tb][:], h, tb)
                    gathers[tb] = q_gather(tb)
                    if tb >= 2:
                        q_mul(tb - 2, bcs_pend.pop(tb - 2))
                    if tb >= 1:
                        q_chain(tb - 1, gathers.pop(tb - 1))
                q_chain(TB - 1, gathers.pop(TB - 1))
                for tb in (TB - 2, TB - 1):
                    q_mul(tb, q_bc_issue(tb))

            # ------------- phase B+C: attention + out projection ----------
            atp = outer.enter_context(tc.tile_pool(name="atp", bufs=3))
            accp = outer.enter_context(tc.tile_pool(name="accp", bufs=3))
            redp = outer.enter_context(tc.tile_pool(name="redp", bufs=2))
            bcy = outer.enter_context(tc.tile_pool(name="bcy", bufs=2))
            invp = outer.enter_context(tc.tile_pool(name="invp", bufs=2))
            wo_pool = outer.enter_context(tc.tile_pool(name="wo", bufs=1))
            ostage = outer.enter_context(tc.tile_pool(name="ostage", bufs=3))
            ps_st = outer.enter_context(
                tc.tile_pool(name="ps_st", bufs=4, space="PSUM"))
            ps_y = outer.enter_context(
                tc.tile_pool(name="ps_y", bufs=2, space="PSUM"))
            ps_o = outer.enter_context(
                tc.tile_pool(name="ps_o", bufs=2, space="PSUM"))

            wo_sb = wo_pool.tile([P, HPG * D], MM_DT, tag="wo")
            nc.sync.dma_start(wo_sb[:], _ld(wot[:]))

            cols = [(qb, h) for qb in range(TB) for h in range(HPG)]
            state = {}   # (qb, h) -> (strip, ps_yt slot deferred)

            def live_w(qb, kt):
                """Live (causal) column count of k-tile kt in q-block qb."""
                j = kt - 4 * qb
                return 512 - 128 * j if j > 0 else 512

            f16 = mybir.dt.float16

            def s1(qb, h):
                """Scores (+causal bias) -> exp -> pre-masked strip; the
                k-rowsum accumulates on the DVE in fp16 alongside."""
                n_k = 4 * (qb + 1)
                qh = qts[h][qb][:]
                strip = atp.tile([P, NT * 512], MM_DT, tag="strip")
                acc = accp.tile([P, 512], f16, tag="acc")
                for kt in range(n_k):
                    w = live_w(qb, kt)
                    c0 = 512 - w
                    diag = kt - 4 * qb >= 0
                    st = ps_st.tile([P, 512], f32, tag="st")
                    nc.tensor.matmul(
                        st[:, c0:512],
                        kt_sb[:, kt * P:(kt + 1) * P],
                        qh[:, c0:512], start=True, stop=not diag)
                    if diag:  # add -30 on the causal triangle via identity
                        nc.tensor.matmul(
                            st[:, c0:512], identb[:],
                            trimat[:, 384:384 + w], start=False, stop=True)
                    ssl = strip[:, kt * 512 + c0:(kt + 1) * 512]
                    nc.scalar.activation(ssl, st[:, c0:512], AF.Exp)
                    if kt == 0:
                        nc.vector.tensor_copy(acc[:], ssl)
                    else:
                        nc.vector.tensor_add(acc[:, c0:512],
                                             acc[:, c0:512], ssl)
                state[(qb, h)] = (strip, acc)

            def s2(qb, h):
                """Partition-reduce the rowsums, AV, normalize."""
                n_k = 4 * (qb + 1)
                strip, acc = state.pop((qb, h))
                red = redp.tile([P, 512], f16, tag="red")
                nc.gpsimd.partition_all_reduce(
                    red[:], acc[:], channels=P,
                    reduce_op=bass_isa.ReduceOp.add)
                sf = invp.tile([1, 512], f32, tag="sf")
                nc.scalar.activation(sf[:], red[0:1, :], AF.Copy)
                inv_row = invp.tile([1, 512], f32, tag="inv")
                nc.vector.reciprocal_approx_fast(inv_row[:], sf[:])
                ps_yt = ps_y.tile([P, 512], f32, tag="y")
                for kt in range(n_k):
                    w = live_w(qb, kt)
                    c0 = 512 - w
                    nc.tensor.matmul(
                        ps_yt[:, c0:512], vtm_sb[:, kt * P:(kt + 1) * P],
                        strip[:, kt * 512 + c0:(kt + 1) * 512],
                        start=(kt == 0), stop=(kt == n_k - 1))
                yslice = yts[h][qb][:]
                nc.vector.tensor_copy(yslice, ps_yt[:])
                bc = bcy.tile([P, 512], f32, tag="bcy")
                nc.gpsimd.partition_broadcast(bc[:], inv_row[:])
                nc.vector.tensor_mul(yslice, yslice, bc[:])

            def outproj(tb):
                for ot in range(NT):
                    ps = ps_o.tile([P, 512], f32, tag="o")
                    for h in range(HPG):
                        nc.tensor.matmul(
                            ps[:],
                            wo_sb[:, h * D + ot * P:h * D + (ot + 1) * P],
                            yts[h][tb][:],
                            start=(h == 0), stop=(h == HPG - 1))
                    o_sb = ostage.tile([P, 512], bf16, tag="osb")
                    nc.any.tensor_copy(o_sb[:], ps[:])
                    nc.sync.dma_start(
                        outT[ot * P:(ot + 1) * P, tb * 512:(tb + 1) * 512],
                        o_sb[:])

            # software pipeline: s1 runs two columns ahead of s2 so the
            # exp + rowsum-reduce chain always has ~2 columns of slack;
            # each t-block's out-projection lands one slot after its
            # last s2.
            pending_out = None
            for i in range(len(cols) + 2):
                if i < len(cols):
                    s1(*cols[i])
                if i >= 2:
                    qb_d, h_d = cols[i - 2]
                    s2(qb_d, h_d)
                    if pending_out is not None:
                        outproj(pending_out)
                        pending_out = None
                    if h_d == HPG - 1:
                        pending_out = qb_d
            if pending_out is not None:
                outproj(pending_out)

    nc.compile()
    return nc


def make_in_maps(q, k, v, Wq, Wk, Wv, Wo, g):
    import ml_dtypes
    st = ml_dtypes.bfloat16 if ST_DT == bf16 else np.float32
    in_maps = []
    act_t = {}
    for b in range(B):
        qTb = np.ascontiguousarray(q[b].T).astype(st)
        # [TB, P, ND*512]: row p of block tb = concat_n qT[n*128+p, tb*512:]
        qTt = np.ascontiguousarray(
            qTb.reshape(ND, P, TB, 512).transpose(2, 1, 0, 3)
            .reshape(TB, P, ND * 512))
        act_t[b] = (
            qTt,
            np.ascontiguousarray(k[b].T).astype(st),
            np.ascontiguousarray(v[b].T).astype(st),
        )

    def wtile(wT, cols):  # wT: (D, cols) -> [P, ND*cols] row-tiled
        return np.ascontiguousarray(
            np.ascontiguousarray(wT).reshape(-1, P, cols)
            .transpose(1, 0, 2).reshape(P, -1)).astype(st)

    g_flat = np.asarray(g, dtype=np.float32).reshape(H)
    for c in range(8):
        b, gi = divmod(c, KVH)
        qTt, kTb, vTb = act_t[b]
        e0 = gi * E
        gvals = g_flat[gi * HPG:(gi + 1) * HPG] / math.sqrt(DK)
        gs_wide = np.repeat(gvals, P)  # [HPG*P], per-head gain replicated
        in_maps.append({
            "qTt": qTt, "kT": kTb, "vT": vTb,
            "wqt": wtile(Wq[e0:e0 + E, :].T, E),
            "wkt": wtile(Wk[gi * DK:(gi + 1) * DK, :].T, DK),
            "wvt": wtile(Wv[gi * DK:(gi + 1) * DK, :].T, DK),
            "wot": wtile(Wo[:, e0:e0 + E].T, D),
            "gs16": np.broadcast_to(gs_wide[None, :], (NT, HPG * P)).copy(),
        })
    return in_maps


_cached = {}


def kernel(q, k, v, Wq, Wk, Wv, Wo, g, _trace=False, _tmpdir=None):
    if "nc" not in _cached:
        _cached["nc"] = build_kernel()
    nc = _cached["nc"]
    in_maps = make_in_maps(
        np.asarray(q, np.float32), np.asarray(k, np.float32),
        np.asarray(v, np.float32), np.asarray(Wq, np.float32),
        np.asarray(Wk, np.float32), np.asarray(Wv, np.float32),
        np.asarray(Wo, np.float32), g)
    res = run_bass_kernel_spmd(
        nc, in_maps, list(range(8)), trace=_trace, tmpdir=_tmpdir)
    out = np.empty((B, T, D), dtype=np.float32)
    for b in range(B):
        acc = res.results[4 * b]["outT"].astype(np.float32)
        for gi in range(1, KVH):
            acc += res.results[4 * b + gi]["outT"].astype(np.float32)
        out[b] = acc.T
    kernel.last_results = res
    return out


# revision 47
# speedup vs baseline: 1.0082x; 1.0082x over previous
"""GQA attention with QK-norm for Trainium2, sharded over 8 NeuronCores.

Problem: B=2, T=2048, D=2048, H=16 query heads, KVH=4 kv heads, dk=128.
    Q = q @ Wq.T ; K = k @ Wk.T ; V = v @ Wv.T  (per batch)
    Q = g * l2norm(Q, per head) ; K = l2norm(K, per head)
    out = softmax(causal(Q K^T / sqrt(dk))) V @ Wo.T

Sharding: core c = 4*b + gi handles batch b and kv-head group gi
(4 query heads + 1 kv head). Each core computes a row-shard of the
output projection (O^T partial over its 512 head-dims); the host sums
the 4 partials per batch. No device collectives.

On-core layout is feature-major ("transposed") throughout; softmax
numerator exp(S^T) needs no max-subtraction because QK-norm bounds
|scores| <= g/sqrt(dk) ~ 0.674.

v2 schedule (PE-density rewrite):
  phase A: K-proj -> K-norm -> Q-proj (tb-outer) -> Q-norms -> V-proj
  -> V-transpose.  Norm chains hide under the next projection's
  matmuls; broadcasts use gpsimd partition_broadcast (no HBM hop).
  phase B: (qb, h) columns are software-pipelined one head ahead:
  PE stream is s1(i), s1(i+1), s2(i), s1(i+2), s2(i+1), ... where
  s1 = scores matmuls (exp on ACT trails one head behind) and
  s2 = rowsum + AV matmuls off the finished exp strip, so the PE
  never waits on the scalar engine.  Diagonal k-tiles are narrowed
  to their live (causal) columns in scores/rowsum/AV/exp.  The
  out-projection of each t-block is emitted two pipeline slots after
  the block's last head, hiding the rowsum-reciprocal chain.
"""

import math
import os
import sys

for _p in ("/opt/trn_rl_repo",):
    if _p not in sys.path:
        sys.path.append(_p)

import numpy as np
from concourse import bacc, bass_isa, mybir, tile
from concourse.bass_utils import run_bass_kernel_spmd
from concourse.masks import make_identity

B, T, D, H, KVH, DK = 2, 2048, 2048, 16, 4, 128
HPG = H // KVH          # query heads per core (group)
E = HPG * DK            # 512: q-head dims per core
P = 128
TB = 4                  # t blocks of 512
NT = T // P             # 16 tiles of 128 along T
ND = D // P             # 16 contraction tiles
f32 = mybir.dt.float32
f32r = mybir.dt.float32r
bf16 = mybir.dt.bfloat16
AF = mybir.ActivationFunctionType
EPS2 = 1e-24

MM_DT = {"f32r": f32r, "bf16": bf16, "f32": f32}[
    os.environ.get("ATTN_DT", "bf16")]
ST_DT = bf16 if MM_DT == bf16 else f32


def _ld(x):
    return x.bitcast(MM_DT) if MM_DT is f32r else x


def build_kernel():
    nc = bacc.Bacc(None, target_bir_lowering=False)

    # host-pre-tiled inputs (see make_in_maps): every DMA below reads
    # contiguous multi-KB rows.
    qTt = nc.declare_dram_parameter("qTt", [TB, P, ND * 512], ST_DT,
                                    isOutput=False)
    kT = nc.declare_dram_parameter("kT", [D, T], ST_DT, isOutput=False)
    vT = nc.declare_dram_parameter("vT", [D, T], ST_DT, isOutput=False)
    wqt = nc.declare_dram_parameter("wqt", [P, ND * E], ST_DT, isOutput=False)
    wkt = nc.declare_dram_parameter("wkt", [P, ND * DK], ST_DT,
                                    isOutput=False)
    wvt = nc.declare_dram_parameter("wvt", [P, ND * DK], ST_DT,
                                    isOutput=False)
    wot = nc.declare_dram_parameter("wot", [P, HPG * D], ST_DT,
                                    isOutput=False)
    gs16 = nc.declare_dram_parameter("gs16", [NT, HPG * P], f32,
                                     isOutput=False)
    outT = nc.declare_dram_parameter("outT", [D, T], bf16, isOutput=True)

    n2_dram = nc.dram_tensor("n2_dram", [HPG + 1, T], f32)
    c_dram = nc.dram_tensor("c_dram", [HPG + 1, T], bf16)

    from contextlib import ExitStack

    with tile.TileContext(nc) as tc:
        with ExitStack() as outer:
            const = outer.enter_context(tc.tile_pool(name="const", bufs=1))
            persist = outer.enter_context(tc.tile_pool(name="persist", bufs=1))

            ident = const.tile([P, P], f32, tag="ident")
            make_identity(nc, ident[:])
            ones_f32 = const.tile([P, 1], f32, tag="ones_f32")
            nc.vector.memset(ones_f32[:], 1.0)
            ones = const.tile([P, 1], MM_DT, tag="ones")
            nc.vector.tensor_copy(ones[:], ones_f32[:])
            ones16 = const.tile([P, 1], mybir.dt.float16, tag="ones16")
            nc.vector.tensor_copy(ones16[:], ones_f32[:])
            gs_sb = const.tile([NT, HPG * P], f32, tag="gs")
            nc.sync.dma_start(gs_sb[:], gs16[:])
            eps16 = const.tile([NT, 1], f32, tag="eps16")
            nc.vector.memset(eps16[:], EPS2)
            # causal additive mask: M[p, c] = 0 iff c >= p + 384 else -30;
            # added to the scores psum via an identity matmul so exp()
            # produces an already-masked strip. Live part of diagonal
            # k-tile: M[:, 384 : 384+W].
            identb = const.tile([P, P], MM_DT, tag="identb")
            nc.vector.tensor_copy(identb[:], ident[:])
            trimat = const.tile([P, 896], MM_DT, tag="trimat")
            nc.vector.memset(trimat[:], 0.0)
            nc.gpsimd.affine_select(
                out=trimat[:], in_=trimat[:],
                compare_op=mybir.AluOpType.is_ge,
                fill=-30.0, base=-384,
                pattern=[[1, 896]], channel_multiplier=-1,
            )

            # per-(head, t-block) tiles: slice-precise dependencies so a
            # late norm-scale write never false-blocks unrelated readers
            qts = [[persist.tile([P, 512], MM_DT, tag=f"qt{h}_{tb}",
                                 name=f"qt{h}_{tb}")
                    for tb in range(TB)] for h in range(HPG)]
            kt_sb = persist.tile([P, T], MM_DT, tag="kt")
            vtm_sb = persist.tile([P, T], MM_DT, tag="vtm")
            yts = [[persist.tile([P, 512], MM_DT, tag=f"yt{h}_{tb}",
                                 name=f"yt{h}_{tb}")
                    for tb in range(TB)] for h in range(HPG)]

            # ---------------- phase A: projections + norms ----------------
            with ExitStack() as pa:
                wpool = pa.enter_context(tc.tile_pool(name="wpool", bufs=1))
                actsq = pa.enter_context(tc.tile_pool(name="actsq", bufs=2))
                actskv = pa.enter_context(tc.tile_pool(name="actskv", bufs=6))
                actsv = pa.enter_context(tc.tile_pool(name="actsv", bufs=5))
                scratch = pa.enter_context(tc.tile_pool(name="scratch",
                                                        bufs=2))
                bcast = pa.enter_context(tc.tile_pool(name="bcast", bufs=8))
                bckp = pa.enter_context(tc.tile_pool(name="bckp", bufs=1))
                rows = pa.enter_context(tc.tile_pool(name="rows", bufs=4))
                smal = pa.enter_context(tc.tile_pool(name="smal", bufs=2))
                psA = pa.enter_context(
                    tc.tile_pool(name="psA", bufs=6, space="PSUM"))
                psTP = pa.enter_context(
                    tc.tile_pool(name="psTP", bufs=2, space="PSUM"))

                def proj_kv(src_dram, w_sb, dst_fn, pool):
                    # k/v chunks alternate between the sync and scalar HWDGE
                    # queues so the stream pulls at 2x single-queue rate.
                    accs = [psA.tile([P, 512], f32, tag="proj",
                                     name=f"acc{_t}") for _t in range(TB)]
                    for n in range(ND):
                        a = pool.tile([P, T], MM_DT, tag="akv")
                        eng = nc.sync if n % 2 == 0 else nc.scalar
                        eng.dma_start(
                            a[:], _ld(src_dram[n * P:(n + 1) * P, :]))
                        for tb in range(TB):
                            nc.tensor.matmul(
                                accs[tb][:],
                                w_sb[:, n * DK:(n + 1) * DK],
                                a[:, tb * 512:(tb + 1) * 512],
                                start=(n == 0), stop=(n == ND - 1))
                    for tb in range(TB):
                        dst_fn(tb, accs[tb])

                def norm_rowsums(xt_tb, idx, tb):
                    """Partition sums of xt_tb^2 [128, 512] via a ones-matmul,
                    staged to n2_dram for the per-head tail."""
                    sq = scratch.tile([P, 512], MM_DT, tag="sqs")
                    nc.vector.tensor_mul(sq[:], xt_tb, xt_tb)
                    ps = psTP.tile([1, 512], f32, tag="tp")
                    nc.tensor.matmul(ps[:], ones[:], sq[:],
                                     start=True, stop=True)
                    n2row = rows.tile([1, 512], f32, tag="n2row")
                    nc.vector.tensor_copy(n2row[:], ps[:])
                    nc.sync.dma_start(
                        n2_dram[idx:idx + 1, tb * 512:(tb + 1) * 512],
                        n2row[:])

                def rsqrt_chain(n2c, shp, gs_ap):
                    """y = rsqrt(n2c) (* gain), one Newton step, bf16 out."""
                    sq_c = smal.tile(shp, f32, tag="sqc")
                    nc.scalar.activation(sq_c[:], n2c[:], AF.Sqrt,
                                         bias=eps16[0:shp[0], :])
                    y0 = smal.tile(shp, f32, tag="y0")
                    nc.vector.reciprocal_approx_fast(y0[:], sq_c[:])
                    t1 = smal.tile(shp, f32, tag="t1")
                    nc.vector.tensor_mul(t1[:], y0[:], y0[:])
                    nc.vector.tensor_mul(t1[:], t1[:], n2c[:])
                    nc.vector.tensor_scalar(
                        out=t1[:], in0=t1[:], scalar1=-0.5, scalar2=1.5,
                        op0=mybir.AluOpType.mult, op1=mybir.AluOpType.add)
                    nc.vector.tensor_mul(y0[:], y0[:], t1[:])
                    if gs_ap is not None:
                        nc.vector.tensor_mul(y0[:], y0[:], gs_ap)
                    y0c = smal.tile(shp, bf16, tag="y0c")
                    nc.vector.tensor_copy(y0c[:], y0[:])
                    return y0c

                # --- Q-norm tail, software-pipelined per t-block ---------
                def q_gather(tb):
                    n2c = smal.tile([TB, HPG * P], f32, tag="n2c")
                    for h in range(HPG):
                        nc.sync.dma_start(
                            n2c[:, h * P:(h + 1) * P],
                            n2_dram[h, tb * 512:(tb + 1) * 512]
                            .rearrange("(c p) -> c p", p=P))
                    return n2c

                def q_chain(tb, n2c):
                    # gain is per-head (same for every chunk row): rows 0:4
                    y0c = rsqrt_chain(n2c, [TB, HPG * P], gs_sb[0:4, :])
                    for h in range(HPG):
                        nc.sync.dma_start(
                            c_dram[h, tb * 512:(tb + 1) * 512]
                            .rearrange("(c p) -> c p", p=P),
                            y0c[:, h * P:(h + 1) * P])

                def q_bc_issue(tb):
                    bcs = []
                    for h in range(HPG):
                        bc = bcast.tile([P, 512], bf16, tag="bc")
                        nc.sync.dma_start(
                            bc[:], c_dram[h:h + 1, tb * 512:(tb + 1) * 512]
                            .to_broadcast((P, 512)))
                        bcs.append(bc)
                    return bcs

                def q_mul(tb, bcs):
                    for h in range(HPG):
                        nc.vector.tensor_mul(qts[h][tb][:], qts[h][tb][:],
                                             bcs[h][:])

                # K first: wk rides the scalar queue while the first kT
                # chunk rides sync, so both land immediately; its norm
                # chain hides under the V projection.
                wk_sb = wpool.tile([P, ND * DK], MM_DT, tag="wk")
                nc.scalar.dma_start(wk_sb[:], _ld(wkt[:]))
                proj_kv(kT, wk_sb,
                        lambda tb, ps: nc.any.tensor_copy(
                            kt_sb[:, tb * 512:(tb + 1) * 512], ps[:]),
                        actskv)
                for tb in range(TB):
                    norm_rowsums(kt_sb[:, tb * 512:(tb + 1) * 512], HPG, tb)
                n2k = smal.tile([NT, P], f32, tag="n2c")
                nc.sync.dma_start(
                    n2k[:], n2_dram[HPG, :].rearrange("(c p) -> c p", p=P))
                y0k = rsqrt_chain(n2k, [NT, P], None)
                nc.sync.dma_start(
                    c_dram[HPG, :].rearrange("(c p) -> c p", p=P), y0k[:])
                bck = bckp.tile([P, T], bf16, tag="bck")
                nc.sync.dma_start(
                    bck[:], c_dram[HPG:HPG + 1, :].to_broadcast((P, T)))
                nc.vector.tensor_mul(kt_sb[:], kt_sb[:], bck[:])

                # V second: transposed per t-block as each psum drains.
                wv_sb = wpool.tile([P, ND * DK], MM_DT, tag="wv")
                nc.scalar.dma_start(wv_sb[:], _ld(wvt[:]))
                # wq prefetched during the V projection
                wq_sb = wpool.tile([P, ND * E], MM_DT, tag="wq")
                nc.sync.dma_start(wq_sb[:], _ld(wqt[:]))

                def v_stage(tb, ps):
                    vs = scratch.tile([P, 512], f32, tag="vstg")
                    nc.any.tensor_copy(vs[:], ps[:])
                    for j in range(4):
                        n = 4 * tb + j
                        tp = psTP.tile([P, P], f32, tag="tp")
                        nc.tensor.transpose(
                            tp[:], vs[:, j * P:(j + 1) * P], ident[:])
                        nc.vector.tensor_copy(
                            vtm_sb[:, n * P:(n + 1) * P], tp[:])
                proj_kv(vT, wv_sb, v_stage, actsv)

                # Q last, tb-outer; per-tb norm tails pipelined 1-2 blocks
                # behind so every chain input has its DMA latency covered.
                NH = ND // 2
                gathers, bcs_pend = {}, {}
                for tb in range(TB):
                    if tb >= 2:
                        bcs_pend[tb - 2] = q_bc_issue(tb - 2)
                    accs = [psA.tile([P, 512], f32, tag="proj",
                                     name=f"qacc{_h}") for _h in range(HPG)]
                    for half in range(2):
                        a = actsq.tile([P, NH * 512], MM_DT, tag="acts")
                        nc.sync.dma_start(
                            a[:], _ld(qTt[tb, :, half * NH * 512:
                                          (half + 1) * NH * 512]))
                        for h in range(HPG):
                            for nn in range(NH):
                                n = half * NH + nn
                                nc.tensor.matmul(
                                    accs[h][:],
                                    wq_sb[:,
                                          n * E + h * P:n * E + (h + 1) * P],
                                    a[:, nn * 512:(nn + 1) * 512],
                                    start=(n == 0), stop=(n == ND - 1))
                    for h in range(HPG):
                        nc.vector.tensor_copy(qts[h][tb][:], accs[h][:])
                        norm_rowsums(qts[h][tb][:], h, tb)
                    gathers[tb] = q_gather(tb)
                    if tb >= 2:
                        q_mul(tb - 2, bcs_pend.pop(tb - 2))
                    if tb >= 1:
                        q_chain(tb - 1, gathers.pop(tb - 1))
                q_chain(TB - 1, gathers.pop(TB - 1))
                for tb in (TB - 2, TB - 1):
                    q_mul(tb, q_bc_issue(tb))

            # ------------- phase B+C: attention + out projection ----------
            atp = outer.enter_context(tc.tile_pool(name="atp", bufs=3))
            accp = outer.enter_context(tc.tile_pool(name="accp", bufs=3))
            redp = outer.enter_context(tc.tile_pool(name="redp", bufs=2))
            bcy = outer.enter_context(tc.tile_pool(name="bcy", bufs=2))
            invp = outer.enter_context(tc.tile_pool(name="invp", bufs=2))
            wo_pool = outer.enter_context(tc.tile_pool(name="wo", bufs=1))
            ostage = outer.enter_context(tc.tile_pool(name="ostage", bufs=3))
            ps_st = outer.enter_context(
                tc.tile_pool(name="ps_st", bufs=3, space="PSUM"))
            ps_sums = outer.enter_context(
                tc.tile_pool(name="ps_sums", bufs=1, space="PSUM"))
            ps_y = outer.enter_context(
                tc.tile_pool(name="ps_y", bufs=2, space="PSUM"))
            ps_o = outer.enter_context(
                tc.tile_pool(name="ps_o", bufs=2, space="PSUM"))

            wo_sb = wo_pool.tile([P, HPG * D], MM_DT, tag="wo")
            nc.sync.dma_start(wo_sb[:], _ld(wot[:]))

            # largest q-block first: its 16-tile s1 columns cover the
            # exp/rowsum pipeline-fill latency at attention start
            cols = [(qb, h) for qb in range(TB - 1, -1, -1)
                    for h in range(HPG)]
            state = {}   # (qb, h) -> (strip, ps_yt slot deferred)

            def live_w(qb, kt):
                """Live (causal) column count of k-tile kt in q-block qb."""
                j = kt - 4 * qb
                return 512 - 128 * j if j > 0 else 512

            f16 = mybir.dt.float16

            def s1(qb, h):
                """Scores (+causal bias) -> exp -> pre-masked strip; the
                k-rowsum accumulates on the DVE in fp16 alongside."""
                n_k = 4 * (qb + 1)
                qh = qts[h][qb][:]
                strip = atp.tile([P, NT * 512], MM_DT, tag="strip")
                acc = accp.tile([P, 512], f16, tag="acc")
                for kt in range(n_k):
                    w = live_w(qb, kt)
                    c0 = 512 - w
                    diag = kt - 4 * qb >= 0
                    st = ps_st.tile([P, 512], f32, tag="st")
                    nc.tensor.matmul(
                        st[:, c0:512],
                        kt_sb[:, kt * P:(kt + 1) * P],
                        qh[:, c0:512], start=True, stop=not diag)
                    if diag:  # add -30 on the causal triangle via identity
                        nc.tensor.matmul(
                            st[:, c0:512], identb[:],
                            trimat[:, 384:384 + w], start=False, stop=True)
                    ssl = strip[:, kt * 512 + c0:(kt + 1) * 512]
                    nc.scalar.activation(ssl, st[:, c0:512], AF.Exp)
                    if kt == 0:
                        nc.vector.tensor_copy(acc[:], ssl)
                    else:
                        nc.vector.tensor_add(acc[:, c0:512],
                                             acc[:, c0:512], ssl)
                state[(qb, h)] = (strip, acc)

            def s2(qb, h):
                """Partition-reduce the rowsums (one ones-matmul), AV,
                normalize."""
                n_k = 4 * (qb + 1)
                strip, acc = state.pop((qb, h))
                ps_sm = ps_sums.tile([1, 512], f32, tag="sums")
                nc.tensor.matmul(ps_sm[:], ones16[:], acc[:],
                                 start=True, stop=True)
                inv_row = invp.tile([1, 512], f32, tag="inv")
                nc.vector.reciprocal_approx_fast(inv_row[:], ps_sm[:])
                ps_yt = ps_y.tile([P, 512], f32, tag="y")
                for kt in range(n_k):
                    w = live_w(qb, kt)
                    c0 = 512 - w
                    nc.tensor.matmul(
                        ps_yt[:, c0:512], vtm_sb[:, kt * P:(kt + 1) * P],
                        strip[:, kt * 512 + c0:(kt + 1) * 512],
                        start=(kt == 0), stop=(kt == n_k - 1))
                yslice = yts[h][qb][:]
                nc.vector.tensor_copy(yslice, ps_yt[:])
                bc = bcy.tile([P, 512], f32, tag="bcy")
                nc.gpsimd.partition_broadcast(bc[:], inv_row[:])
                nc.vector.tensor_mul(yslice, yslice, bc[:])

            def outproj(tb):
                for ot in range(NT):
                    ps = ps_o.tile([P, 512], f32, tag="o")
                    for h in range(HPG):
                        nc.tensor.matmul(
                            ps[:],
                            wo_sb[:, h * D + ot * P:h * D + (ot + 1) * P],
                            yts[h][tb][:],
                            start=(h == 0), stop=(h == HPG - 1))
                    o_sb = ostage.tile([P, 512], bf16, tag="osb")
                    nc.any.tensor_copy(o_sb[:], ps[:])
                    nc.sync.dma_start(
                        outT[ot * P:(ot + 1) * P, tb * 512:(tb + 1) * 512],
                        o_sb[:])

            # software pipeline: s1 runs two columns ahead of s2 so the
            # exp + rowsum chain always has ~2 columns of slack; each
            # t-block's out-projection lands two slots after its last s2.
            pending = []   # (emit_at_i, tb)
            for i in range(len(cols) + 2):
                if i < len(cols):
                    s1(*cols[i])
                if i >= 2:
                    qb_d, h_d = cols[i - 2]
                    s2(qb_d, h_d)
                    if h_d == HPG - 1:
                        pending.append([i + 2, qb_d])
                while pending and pending[0][0] <= i:
                    outproj(pending.pop(0)[1])
            for _, tb in pending:
                outproj(tb)

    nc.compile()
    return nc


def make_in_maps(q, k, v, Wq, Wk, Wv, Wo, g):
    import ml_dtypes
    st = ml_dtypes.bfloat16 if ST_DT == bf16 else np.float32
    in_maps = []
    act_t = {}
    for b in range(B):
        qTb = np.ascontiguousarray(q[b].T).astype(st)
        # [TB, P, ND*512]: row p of block tb = concat_n qT[n*128+p, tb*512:]
        qTt = np.ascontiguousarray(
            qTb.reshape(ND, P, TB, 512).transpose(2, 1, 0, 3)
            .reshape(TB, P, ND * 512))
        act_t[b] = (
            qTt,
            np.ascontiguousarray(k[b].T).astype(st),
            np.ascontiguousarray(v[b].T).astype(st),
        )

    def wtile(wT, cols):  # wT: (D, cols) -> [P, ND*cols] row-tiled
        return np.ascontiguousarray(
            np.ascontiguousarray(wT).reshape(-1, P, cols)
            .transpose(1, 0, 2).reshape(P, -1)).astype(st)

    g_flat = np.asarray(g, dtype=np.float32).reshape(H)
    for c in range(8):
        b, gi = divmod(c, KVH)
        qTt, kTb, vTb = act_t[b]
        e0 = gi * E
        gvals = g_flat[gi * HPG:(gi + 1) * HPG] / math.sqrt(DK)
        gs_wide = np.repeat(gvals, P)  # [HPG*P], per-head gain replicated
        in_maps.append({
            "qTt": qTt, "kT": kTb, "vT": vTb,
            "wqt": wtile(Wq[e0:e0 + E, :].T, E),
            "wkt": wtile(Wk[gi * DK:(gi + 1) * DK, :].T, DK),
            "wvt": wtile(Wv[gi * DK:(gi + 1) * DK, :].T, DK),
            "wot": wtile(Wo[:, e0:e0 + E].T, D),
            "gs16": np.broadcast_to(gs_wide[None, :], (NT, HPG * P)).copy(),
        })
    return in_maps


_cached = {}


def kernel(q, k, v, Wq, Wk, Wv, Wo, g, _trace=False, _tmpdir=None):
    if "nc" not in _cached:
        _cached["nc"] = build_kernel()
    nc = _cached["nc"]
    in_maps = make_in_maps(
        np.asarray(q, np.float32), np.asarray(k, np.float32),
        np.asarray(v, np.float32), np.asarray(Wq, np.float32),
        np.asarray(Wk, np.float32), np.asarray(Wv, np.float32),
        np.asarray(Wo, np.float32), g)
    res = run_bass_kernel_spmd(
        nc, in_maps, list(range(8)), trace=_trace, tmpdir=_tmpdir)
    out = np.empty((B, T, D), dtype=np.float32)
    for b in range(B):
        acc = res.results[4 * b]["outT"].astype(np.float32)
        for gi in range(1, KVH):
            acc += res.results[4 * b + gi]["outT"].astype(np.float32)
        out[b] = acc.T
    kernel.last_results = res
    return out


# revision 49
# speedup vs baseline: 1.0778x; 1.0690x over previous
"""GQA attention with QK-norm for Trainium2, sharded over 8 NeuronCores.

Problem: B=2, T=2048, D=2048, H=16 query heads, KVH=4 kv heads, dk=128.
    Q = q @ Wq.T ; K = k @ Wk.T ; V = v @ Wv.T  (per batch)
    Q = g * l2norm(Q, per head) ; K = l2norm(K, per head)
    out = softmax(causal(Q K^T / sqrt(dk))) V @ Wo.T

Sharding: core c = 4*b + gi handles batch b and kv-head group gi
(4 query heads + 1 kv head). Each core computes a row-shard of the
output projection (O^T partial over its 512 head-dims); the host sums
the 4 partials per batch. No device collectives.

On-core layout is feature-major ("transposed") throughout; softmax
numerator exp(S^T) needs no max-subtraction because QK-norm bounds
|scores| <= g/sqrt(dk) ~ 0.674.

v2 schedule (PE-density rewrite):
  phase A: K-proj -> K-norm -> Q-proj (tb-outer) -> Q-norms -> V-proj
  -> V-transpose.  Norm chains hide under the next projection's
  matmuls; broadcasts use gpsimd partition_broadcast (no HBM hop).
  phase B: (qb, h) columns are software-pipelined one head ahead:
  PE stream is s1(i), s1(i+1), s2(i), s1(i+2), s2(i+1), ... where
  s1 = scores matmuls (exp on ACT trails one head behind) and
  s2 = rowsum + AV matmuls off the finished exp strip, so the PE
  never waits on the scalar engine.  Diagonal k-tiles are narrowed
  to their live (causal) columns in scores/rowsum/AV/exp.  The
  out-projection of each t-block is emitted two pipeline slots after
  the block's last head, hiding the rowsum-reciprocal chain.
"""

import math
import os
import sys

for _p in ("/opt/trn_rl_repo",):
    if _p not in sys.path:
        sys.path.append(_p)

import numpy as np
from concourse import bacc, bass_isa, mybir, tile
from concourse.bass_utils import run_bass_kernel_spmd
from concourse.masks import make_identity

B, T, D, H, KVH, DK = 2, 2048, 2048, 16, 4, 128
HPG = H // KVH          # query heads per core (group)
E = HPG * DK            # 512: q-head dims per core
P = 128
TB = 4                  # t blocks of 512
NT = T // P             # 16 tiles of 128 along T
ND = D // P             # 16 contraction tiles
f32 = mybir.dt.float32
f32r = mybir.dt.float32r
bf16 = mybir.dt.bfloat16
AF = mybir.ActivationFunctionType
EPS2 = 1e-24

MM_DT = {"f32r": f32r, "bf16": bf16, "f32": f32}[
    os.environ.get("ATTN_DT", "bf16")]
ST_DT = bf16 if MM_DT == bf16 else f32


def _ld(x):
    return x.bitcast(MM_DT) if MM_DT is f32r else x


def build_kernel():
    nc = bacc.Bacc(None, target_bir_lowering=False)

    # host-pre-tiled inputs (see make_in_maps): every DMA below reads
    # contiguous multi-KB rows.
    qTt = nc.declare_dram_parameter("qTt", [TB, P, ND * 512], ST_DT,
                                    isOutput=False)
    kT = nc.declare_dram_parameter("kT", [D, T], ST_DT, isOutput=False)
    vT = nc.declare_dram_parameter("vT", [D, T], ST_DT, isOutput=False)
    wqt = nc.declare_dram_parameter("wqt", [P, ND * E], ST_DT, isOutput=False)
    wkt = nc.declare_dram_parameter("wkt", [P, ND * DK], ST_DT,
                                    isOutput=False)
    wvt = nc.declare_dram_parameter("wvt", [P, ND * DK], ST_DT,
                                    isOutput=False)
    wot = nc.declare_dram_parameter("wot", [P, HPG * D], ST_DT,
                                    isOutput=False)
    gs16 = nc.declare_dram_parameter("gs16", [NT, HPG * P], f32,
                                     isOutput=False)
    outT = nc.declare_dram_parameter("outT", [D, T], bf16, isOutput=True)

    n2_dram = nc.dram_tensor("n2_dram", [HPG + 1, T], f32)
    c_dram = nc.dram_tensor("c_dram", [HPG + 1, T], bf16)

    from contextlib import ExitStack

    with tile.TileContext(nc) as tc:
        with ExitStack() as outer:
            const = outer.enter_context(tc.tile_pool(name="const", bufs=1))
            persist = outer.enter_context(tc.tile_pool(name="persist", bufs=1))

            ident = const.tile([P, P], f32, tag="ident")
            make_identity(nc, ident[:])
            ones_f32 = const.tile([P, 1], f32, tag="ones_f32")
            nc.vector.memset(ones_f32[:], 1.0)
            ones = const.tile([P, 1], MM_DT, tag="ones")
            nc.vector.tensor_copy(ones[:], ones_f32[:])
            ones16 = const.tile([P, 1], mybir.dt.float16, tag="ones16")
            nc.vector.tensor_copy(ones16[:], ones_f32[:])
            gs_sb = const.tile([NT, HPG * P], f32, tag="gs")
            nc.sync.dma_start(gs_sb[:], gs16[:])
            eps16 = const.tile([NT, 1], f32, tag="eps16")
            nc.vector.memset(eps16[:], EPS2)
            # causal additive mask: M[p, c] = 0 iff c >= p + 384 else -30;
            # added to the scores psum via an identity matmul so exp()
            # produces an already-masked strip. Live part of diagonal
            # k-tile: M[:, 384 : 384+W].
            identb = const.tile([P, P], MM_DT, tag="identb")
            nc.vector.tensor_copy(identb[:], ident[:])
            trimat = const.tile([P, 896], MM_DT, tag="trimat")
            nc.vector.memset(trimat[:], 0.0)
            nc.gpsimd.affine_select(
                out=trimat[:], in_=trimat[:],
                compare_op=mybir.AluOpType.is_ge,
                fill=-30.0, base=-384,
                pattern=[[1, 896]], channel_multiplier=-1,
            )

            # per-(head, t-block) tiles: slice-precise dependencies so a
            # late norm-scale write never false-blocks unrelated readers
            qts = [[persist.tile([P, 512], MM_DT, tag=f"qt{h}_{tb}",
                                 name=f"qt{h}_{tb}")
                    for tb in range(TB)] for h in range(HPG)]
            kt_sb = persist.tile([P, T], MM_DT, tag="kt")
            vtm_sb = persist.tile([P, T], MM_DT, tag="vtm")
            yts = [[persist.tile([P, 512], MM_DT, tag=f"yt{h}_{tb}",
                                 name=f"yt{h}_{tb}")
                    for tb in range(TB)] for h in range(HPG)]

            # ---------------- phase A: projections + norms ----------------
            with ExitStack() as pa:
                wpool = pa.enter_context(tc.tile_pool(name="wpool", bufs=1))
                actsq = pa.enter_context(tc.tile_pool(name="actsq", bufs=2))
                actskv = pa.enter_context(tc.tile_pool(name="actskv", bufs=6))
                actsv = pa.enter_context(tc.tile_pool(name="actsv", bufs=5))
                scratch = pa.enter_context(tc.tile_pool(name="scratch",
                                                        bufs=2))
                bcast = pa.enter_context(tc.tile_pool(name="bcast", bufs=8))
                bckp = pa.enter_context(tc.tile_pool(name="bckp", bufs=1))
                rows = pa.enter_context(tc.tile_pool(name="rows", bufs=4))
                smal = pa.enter_context(tc.tile_pool(name="smal", bufs=2))
                psA = pa.enter_context(
                    tc.tile_pool(name="psA", bufs=6, space="PSUM"))
                psTP = pa.enter_context(
                    tc.tile_pool(name="psTP", bufs=2, space="PSUM"))

                def proj_kv(src_dram, w_sb, dst_fn, pool):
                    # k/v chunks alternate between the sync and scalar HWDGE
                    # queues so the stream pulls at 2x single-queue rate.
                    accs = [psA.tile([P, 512], f32, tag="proj",
                                     name=f"acc{_t}") for _t in range(TB)]
                    for n in range(ND):
                        a = pool.tile([P, T], MM_DT, tag="akv")
                        eng = nc.sync if n % 2 == 0 else nc.scalar
                        eng.dma_start(
                            a[:], _ld(src_dram[n * P:(n + 1) * P, :]))
                        for tb in range(TB):
                            nc.tensor.matmul(
                                accs[tb][:],
                                w_sb[:, n * DK:(n + 1) * DK],
                                a[:, tb * 512:(tb + 1) * 512],
                                start=(n == 0), stop=(n == ND - 1))
                    for tb in range(TB):
                        dst_fn(tb, accs[tb])

                def norm_rowsums(xt_tb, idx, tb):
                    """Partition sums of xt_tb^2 [128, 512] via a ones-matmul,
                    staged to n2_dram for the per-head tail."""
                    sq = scratch.tile([P, 512], MM_DT, tag="sqs")
                    nc.vector.tensor_mul(sq[:], xt_tb, xt_tb)
                    ps = psTP.tile([1, 512], f32, tag="tp")
                    nc.tensor.matmul(ps[:], ones[:], sq[:],
                                     start=True, stop=True)
                    n2row = rows.tile([1, 512], f32, tag="n2row")
                    nc.vector.tensor_copy(n2row[:], ps[:])
                    nc.sync.dma_start(
                        n2_dram[idx:idx + 1, tb * 512:(tb + 1) * 512],
                        n2row[:])

                def rsqrt_chain(n2c, shp, gs_ap):
                    """y = rsqrt(n2c) (* gain), one Newton step, bf16 out."""
                    sq_c = smal.tile(shp, f32, tag="sqc")
                    nc.scalar.activation(sq_c[:], n2c[:], AF.Sqrt,
                                         bias=eps16[0:shp[0], :])
                    y0 = smal.tile(shp, f32, tag="y0")
                    nc.vector.reciprocal_approx_fast(y0[:], sq_c[:])
                    t1 = smal.tile(shp, f32, tag="t1")
                    nc.vector.tensor_mul(t1[:], y0[:], y0[:])
                    nc.vector.tensor_mul(t1[:], t1[:], n2c[:])
                    nc.vector.tensor_scalar(
                        out=t1[:], in0=t1[:], scalar1=-0.5, scalar2=1.5,
                        op0=mybir.AluOpType.mult, op1=mybir.AluOpType.add)
                    nc.vector.tensor_mul(y0[:], y0[:], t1[:])
                    if gs_ap is not None:
                        nc.vector.tensor_mul(y0[:], y0[:], gs_ap)
                    y0c = smal.tile(shp, bf16, tag="y0c")
                    nc.vector.tensor_copy(y0c[:], y0[:])
                    return y0c

                # --- Q-norm tail, software-pipelined per t-block ---------
                def q_gather(tb):
                    n2c = smal.tile([TB, HPG * P], f32, tag="n2c")
                    for h in range(HPG):
                        nc.sync.dma_start(
                            n2c[:, h * P:(h + 1) * P],
                            n2_dram[h, tb * 512:(tb + 1) * 512]
                            .rearrange("(c p) -> c p", p=P))
                    return n2c

                def q_chain(tb, n2c):
                    # gain is per-head (same for every chunk row): rows 0:4
                    y0c = rsqrt_chain(n2c, [TB, HPG * P], gs_sb[0:4, :])
                    for h in range(HPG):
                        nc.sync.dma_start(
                            c_dram[h, tb * 512:(tb + 1) * 512]
                            .rearrange("(c p) -> c p", p=P),
                            y0c[:, h * P:(h + 1) * P])

                def q_bc_issue(tb):
                    bcs = []
                    for h in range(HPG):
                        bc = bcast.tile([P, 512], bf16, tag="bc")
                        nc.sync.dma_start(
                            bc[:], c_dram[h:h + 1, tb * 512:(tb + 1) * 512]
                            .to_broadcast((P, 512)))
                        bcs.append(bc)
                    return bcs

                def q_mul(tb, bcs):
                    for h in range(HPG):
                        nc.vector.tensor_mul(qts[h][tb][:], qts[h][tb][:],
                                             bcs[h][:])

                # K first: wk rides the scalar queue while the first kT
                # chunk rides sync, so both land immediately; its norm
                # chain hides under the V projection.
                wk_sb = wpool.tile([P, ND * DK], MM_DT, tag="wk")
                nc.scalar.dma_start(wk_sb[:], _ld(wkt[:]))
                proj_kv(kT, wk_sb,
                        lambda tb, ps: nc.any.tensor_copy(
                            kt_sb[:, tb * 512:(tb + 1) * 512], ps[:]),
                        actskv)
                for tb in range(TB):
                    norm_rowsums(kt_sb[:, tb * 512:(tb + 1) * 512], HPG, tb)
                n2k = smal.tile([NT, P], f32, tag="n2c")
                nc.sync.dma_start(
                    n2k[:], n2_dram[HPG, :].rearrange("(c p) -> c p", p=P))
                y0k = rsqrt_chain(n2k, [NT, P], None)
                nc.sync.dma_start(
                    c_dram[HPG, :].rearrange("(c p) -> c p", p=P), y0k[:])
                bck = bckp.tile([P, T], bf16, tag="bck")
                nc.sync.dma_start(
                    bck[:], c_dram[HPG:HPG + 1, :].to_broadcast((P, T)))
                nc.vector.tensor_mul(kt_sb[:], kt_sb[:], bck[:])

                # V second: transposed per t-block as each psum drains.
                wv_sb = wpool.tile([P, ND * DK], MM_DT, tag="wv")
                nc.scalar.dma_start(wv_sb[:], _ld(wvt[:]))
                # wq prefetched during the V projection
                wq_sb = wpool.tile([P, ND * E], MM_DT, tag="wq")
                nc.sync.dma_start(wq_sb[:], _ld(wqt[:]))

                def v_stage(tb, ps):
                    vs = scratch.tile([P, 512], f32, tag="vstg")
                    nc.any.tensor_copy(vs[:], ps[:])
                    for j in range(4):
                        n = 4 * tb + j
                        tp = psTP.tile([P, P], f32, tag="tp")
                        nc.tensor.transpose(
                            tp[:], vs[:, j * P:(j + 1) * P], ident[:])
                        nc.vector.tensor_copy(
                            vtm_sb[:, n * P:(n + 1) * P], tp[:])
                proj_kv(vT, wv_sb, v_stage, actsv)

                # Q last, tb-outer; per-tb norm tails pipelined 1-2 blocks
                # behind so every chain input has its DMA latency covered.
                NH = ND // 2
                # descending tb so the blocks attention consumes first
                # (largest q-block first) are normalized first
                order = list(range(TB - 1, -1, -1))
                gathers, bcs_pend = {}, {}
                for ti, tb in enumerate(order):
                    if ti >= 2:
                        bcs_pend[order[ti - 2]] = q_bc_issue(order[ti - 2])
                    accs = [psA.tile([P, 512], f32, tag="proj",
                                     name=f"qacc{_h}") for _h in range(HPG)]
                    for half in range(2):
                        a = actsq.tile([P, NH * 512], MM_DT, tag="acts")
                        nc.sync.dma_start(
                            a[:], _ld(qTt[tb, :, half * NH * 512:
                                          (half + 1) * NH * 512]))
                        for h in range(HPG):
                            for nn in range(NH):
                                n = half * NH + nn
                                nc.tensor.matmul(
                                    accs[h][:],
                                    wq_sb[:,
                                          n * E + h * P:n * E + (h + 1) * P],
                                    a[:, nn * 512:(nn + 1) * 512],
                                    start=(n == 0), stop=(n == ND - 1))
                    for h in range(HPG):
                        nc.vector.tensor_copy(qts[h][tb][:], accs[h][:])
                        norm_rowsums(qts[h][tb][:], h, tb)
                    gathers[tb] = q_gather(tb)
                    if ti >= 2:
                        q_mul(order[ti - 2], bcs_pend.pop(order[ti - 2]))
                    if ti >= 1:
                        q_chain(order[ti - 1], gathers.pop(order[ti - 1]))
                q_chain(order[-1], gathers.pop(order[-1]))
                for tb in (order[-2], order[-1]):
                    q_mul(tb, q_bc_issue(tb))

            # ------------- phase B+C: attention + out projection ----------
            atp = outer.enter_context(tc.tile_pool(name="atp", bufs=3))
            accp = outer.enter_context(tc.tile_pool(name="accp", bufs=3))
            redp = outer.enter_context(tc.tile_pool(name="redp", bufs=2))
            bcy = outer.enter_context(tc.tile_pool(name="bcy", bufs=2))
            invp = outer.enter_context(tc.tile_pool(name="invp", bufs=2))
            wo_pool = outer.enter_context(tc.tile_pool(name="wo", bufs=1))
            ostage = outer.enter_context(tc.tile_pool(name="ostage", bufs=3))
            ps_st = outer.enter_context(
                tc.tile_pool(name="ps_st", bufs=3, space="PSUM"))
            ps_sums = outer.enter_context(
                tc.tile_pool(name="ps_sums", bufs=1, space="PSUM"))
            ps_y = outer.enter_context(
                tc.tile_pool(name="ps_y", bufs=2, space="PSUM"))
            ps_o = outer.enter_context(
                tc.tile_pool(name="ps_o", bufs=2, space="PSUM"))

            wo_sb = wo_pool.tile([P, HPG * D], MM_DT, tag="wo")
            nc.sync.dma_start(wo_sb[:], _ld(wot[:]))

            # largest q-block first: its 16-tile s1 columns cover the
            # exp/rowsum pipeline-fill latency at attention start
            cols = [(qb, h) for qb in range(TB - 1, -1, -1)
                    for h in range(HPG)]
            state = {}   # (qb, h) -> (strip, ps_yt slot deferred)

            def live_w(qb, kt):
                """Live (causal) column count of k-tile kt in q-block qb."""
                j = kt - 4 * qb
                return 512 - 128 * j if j > 0 else 512

            f16 = mybir.dt.float16

            def s1(qb, h):
                """Scores (+causal bias) -> exp -> pre-masked strip; the
                k-rowsum accumulates on the DVE in fp16 alongside."""
                n_k = 4 * (qb + 1)
                qh = qts[h][qb][:]
                strip = atp.tile([P, NT * 512], MM_DT, tag="strip")
                acc = accp.tile([P, 512], f16, tag="acc")
                for kt in range(n_k):
                    w = live_w(qb, kt)
                    c0 = 512 - w
                    diag = kt - 4 * qb >= 0
                    st = ps_st.tile([P, 512], f32, tag="st")
                    nc.tensor.matmul(
                        st[:, c0:512],
                        kt_sb[:, kt * P:(kt + 1) * P],
                        qh[:, c0:512], start=True, stop=not diag)
                    if diag:  # add -30 on the causal triangle via identity
                        nc.tensor.matmul(
                            st[:, c0:512], identb[:],
                            trimat[:, 384:384 + w], start=False, stop=True)
                    ssl = strip[:, kt * 512 + c0:(kt + 1) * 512]
                    nc.scalar.activation(ssl, st[:, c0:512], AF.Exp)
                    if kt == 0:
                        nc.vector.tensor_copy(acc[:], ssl)
                    else:
                        nc.vector.tensor_add(acc[:, c0:512],
                                             acc[:, c0:512], ssl)
                state[(qb, h)] = (strip, acc)

            def s2(qb, h):
                """Partition-reduce the rowsums (one ones-matmul), AV,
                normalize."""
                n_k = 4 * (qb + 1)
                strip, acc = state.pop((qb, h))
                ps_sm = ps_sums.tile([1, 512], f32, tag="sums")
                nc.tensor.matmul(ps_sm[:], ones16[:], acc[:],
                                 start=True, stop=True)
                inv_row = invp.tile([1, 512], f32, tag="inv")
                nc.vector.reciprocal_approx_fast(inv_row[:], ps_sm[:])
                ps_yt = ps_y.tile([P, 512], f32, tag="y")
                for kt in range(n_k):
                    w = live_w(qb, kt)
                    c0 = 512 - w
                    nc.tensor.matmul(
                        ps_yt[:, c0:512], vtm_sb[:, kt * P:(kt + 1) * P],
                        strip[:, kt * 512 + c0:(kt + 1) * 512],
                        start=(kt == 0), stop=(kt == n_k - 1))
                yslice = yts[h][qb][:]
                nc.vector.tensor_copy(yslice, ps_yt[:])
                bc = bcy.tile([P, 512], f32, tag="bcy")
                nc.gpsimd.partition_broadcast(bc[:], inv_row[:])
                nc.vector.tensor_mul(yslice, yslice, bc[:])

            def outproj(tb):
                for ot in range(NT):
                    ps = ps_o.tile([P, 512], f32, tag="o")
                    for h in range(HPG):
                        nc.tensor.matmul(
                            ps[:],
                            wo_sb[:, h * D + ot * P:h * D + (ot + 1) * P],
                            yts[h][tb][:],
                            start=(h == 0), stop=(h == HPG - 1))
                    o_sb = ostage.tile([P, 512], bf16, tag="osb")
                    nc.any.tensor_copy(o_sb[:], ps[:])
                    nc.sync.dma_start(
                        outT[ot * P:(ot + 1) * P, tb * 512:(tb + 1) * 512],
                        o_sb[:])

            # software pipeline: s1 runs two columns ahead of s2 so the
            # exp + rowsum chain always has ~2 columns of slack; each
            # t-block's out-projection lands two slots after its last s2.
            pending = []   # (emit_at_i, tb)
            for i in range(len(cols) + 2):
                if i < len(cols):
                    s1(*cols[i])
                if i >= 2:
                    qb_d, h_d = cols[i - 2]
                    s2(qb_d, h_d)
                    if h_d == HPG - 1:
                        pending.append([i + 2, qb_d])
                while pending and pending[0][0] <= i:
                    outproj(pending.pop(0)[1])
            for _, tb in pending:
                outproj(tb)

    nc.compile()
    return nc


def make_in_maps(q, k, v, Wq, Wk, Wv, Wo, g):
    import ml_dtypes
    st = ml_dtypes.bfloat16 if ST_DT == bf16 else np.float32
    in_maps = []
    act_t = {}
    for b in range(B):
        qTb = np.ascontiguousarray(q[b].T).astype(st)
        # [TB, P, ND*512]: row p of block tb = concat_n qT[n*128+p, tb*512:]
        qTt = np.ascontiguousarray(
            qTb.reshape(ND, P, TB, 512).transpose(2, 1, 0, 3)
            .reshape(TB, P, ND * 512))
        act_t[b] = (
            qTt,
            np.ascontiguousarray(k[b].T).astype(st),
            np.ascontiguousarray(v[b].T).astype(st),
        )

    def wtile(wT, cols):  # wT: (D, cols) -> [P, ND*cols] row-tiled
        return np.ascontiguousarray(
            np.ascontiguousarray(wT).reshape(-1, P, cols)
            .transpose(1, 0, 2).reshape(P, -1)).astype(st)

    g_flat = np.asarray(g, dtype=np.float32).reshape(H)
    for c in range(8):
        b, gi = divmod(c, KVH)
        qTt, kTb, vTb = act_t[b]
        e0 = gi * E
        gvals = g_flat[gi * HPG:(gi + 1) * HPG] / math.sqrt(DK)
        gs_wide = np.repeat(gvals, P)  # [HPG*P], per-head gain replicated
        in_maps.append({
            "qTt": qTt, "kT": kTb, "vT": vTb,
            "wqt": wtile(Wq[e0:e0 + E, :].T, E),
            "wkt": wtile(Wk[gi * DK:(gi + 1) * DK, :].T, DK),
            "wvt": wtile(Wv[gi * DK:(gi + 1) * DK, :].T, DK),
            "wot": wtile(Wo[:, e0:e0 + E].T, D),
            "gs16": np.broadcast_to(gs_wide[None, :], (NT, HPG * P)).copy(),
        })
    return in_maps


_cached = {}


def kernel(q, k, v, Wq, Wk, Wv, Wo, g, _trace=False, _tmpdir=None):
    if "nc" not in _cached:
        _cached["nc"] = build_kernel()
    nc = _cached["nc"]
    in_maps = make_in_maps(
        np.asarray(q, np.float32), np.asarray(k, np.float32),
        np.asarray(v, np.float32), np.asarray(Wq, np.float32),
        np.asarray(Wk, np.float32), np.asarray(Wv, np.float32),
        np.asarray(Wo, np.float32), g)
    res = run_bass_kernel_spmd(
        nc, in_maps, list(range(8)), trace=_trace, tmpdir=_tmpdir)
    out = np.empty((B, T, D), dtype=np.float32)
    for b in range(B):
        acc = res.results[4 * b]["outT"].astype(np.float32)
        for gi in range(1, KVH):
            acc += res.results[4 * b + gi]["outT"].astype(np.float32)
        out[b] = acc.T
    kernel.last_results = res
    return out
